# revision 49
# baseline (speedup 1.0000x reference)
"""Trainium2 Bass kernel for nn_Attention_23476291240422 (sparse attention:
causal + 128-wide noncausal prefix block; b=4, n=2048, dim=2048, 16 heads,
d=128) distributed across 8 NeuronCores.

v4e design for gapless TensorE (PE sustains ~1.9 GHz; the kernel is row-count
bound at ~1.33M matmul rows/core, so every PE idle ns is pure loss):
- [inner, rows] AllToAll payloads; the [i,d]->[d,i] flip runs on TensorE,
  one transpose per i-tile, staged into per-chunk [P,4,P] buffers so each
  payload store is ONE contiguous 128KB write (SBUF->DRAM stores serialize
  as DIRECT2D on the issuing sequencer at ~56GB/s — write count matters).
- 2 collective phases by query-position: phase A = query chunks 1,3, phase
  B = chunks 0,2. Scope 1 computes stage-1 + ALL phase-A attention, then
  a2a-A fires and hides under scope 2's phase-B (BC) attention. a2a-B then
  hides under stage-4 pass A (lhsA only needs a2a-A). lhs loads are chunked
  DMAs on the scalar (Activation) DGE queue, pinned after the last exp;
  lhsB's loads are emitted only after pass A so nothing queues behind their
  collective wait. Collectives have a ~20us bulk phase that monopolizes all
  16 DMA engines: everything overlapping a collective is preloaded (va
  bufs=5, wb prefetch distance 3) or buffered (s4o bufs=8, fin bufs=6).
- Startup: kt-quartered accumulation for the first s1 chunk — the first
  matmul needs only 1.25MB (first wq+xb quarters), loads interleaved across
  both HWDGE queues in consumption order; bout load deferred to scope 2.
- Stage-4 in 256-col chunks, output stores alternating sync/scalar
  sequencers (one sequencer alone cannot keep up with stage-4's rate).
- Deep software pipelining: phase-A attention of batch b interleaved inside
  stage-1 of batch b+1; scores of the next (batch,head) pair emitted before
  attn@v of the current one so scalar-engine exp latency is hidden; xb
  prefetched one chunk ahead; causal diagonal masks on the idle gpsimd.
- Softmax scale folded into the q columns of w_qkv on the host.
"""
import os
import sys
import types

import numpy as np
import ml_dtypes

import concourse.bass as bass
import concourse.mybir as mybir
import concourse.tile as tile
from concourse.tile import add_dep_helper
from concourse import bacc
from concourse import bass_utils

B, N, DIM = 4, 2048, 2048
HEADS, D, L = 16, 128, 128
W = 8
HPC = HEADS // W          # 2 heads per core
ROWS = B * N              # 8192
RPC = ROWS // W           # 1024 rows per core
SCALE = float(D) ** -0.5
P = 128
KT = DIM // P             # 16
S1CH = 512                # stage-1 seq chunk width
NBC = N // S1CH           # 4 stage-1 chunks per batch
CHW = 512                 # score chunk width
NJT = N // P              # 16 j-tiles
F32, BF16 = mybir.dt.float32, mybir.dt.bfloat16

# attention chunks (512 queries each): phase A = full chunks 1,3
# (it%8 in 4..7 -> dest rows 512-1023); chunks 0,2 split by pp:
# pp 2,3 -> phase B (rows 256-511), pp 0,1 -> phase C (rows 0-255).
PT_NJ = {0: 4, 1: 8, 2: 12, 3: 16}   # j-tiles needed per chunk


def _install_ntff_hook():
    try:
        import antenv.axon_hooks  # noqa: F401
        return
    except ImportError:
        pass
    try:
        import antenv
        from trn_agent_boot.trn_boot import _ntff_profile_via_ctypes
        hook = [_ntff_profile_via_ctypes("/opt/axon/libaxon_pjrt.so")]
        mod = types.ModuleType("antenv.axon_hooks")
        mod.get_axon_ntff_profile_hook = lambda: hook[0]
        mod.set_axon_ntff_profile_hook = lambda h: hook.__setitem__(0, h)
        sys.modules["antenv.axon_hooks"] = mod
        antenv.axon_hooks = mod
    except Exception:
        pass


def build():
    nc = bacc.Bacc("TRN2", target_bir_lowering=False, debug=False, num_devices=W)

    xT = nc.dram_tensor("xT", [DIM, ROWS], BF16, kind="ExternalInput")
    wq = nc.dram_tensor("wq", [DIM, 6 * P], BF16, kind="ExternalInput")  # q0 q1 k0 k1 v0 v1 (q pre-scaled)
    wout = nc.dram_tensor("wout", [DIM, DIM], BF16, kind="ExternalInput")
    bout = nc.dram_tensor("bout", [1, DIM], F32, kind="ExternalInput")
    out = nc.dram_tensor("out", [RPC, DIM], F32, kind="ExternalOutput")

    tri_np = (np.arange(P)[:, None] <= np.arange(P)[None, :]).astype(ml_dtypes.bfloat16)
    tri = nc.inline_tensor(tri_np, name="tri")
    ident = nc.inline_tensor(np.eye(P, dtype=ml_dtypes.bfloat16), name="ident")

    with tile.TileContext(nc) as tc:
        with (
            tc.tile_pool(name="persist", bufs=1) as persist,
            tc.tile_pool(name="dram", bufs=1, space="DRAM") as dram,
            tc.tile_pool(name="qk", bufs=1) as qkpool,
            tc.tile_pool(name="va", bufs=5) as vapool,
            tc.tile_pool(name="sm", bufs=8) as smpool,
            tc.tile_pool(name="fin", bufs=6) as finpool,
            tc.tile_pool(name="pss", bufs=2, space="PSUM") as psspool,
            tc.tile_pool(name="att", bufs=2, space="PSUM") as attpool,
            tc.tile_pool(name="s2tp", bufs=2, space="PSUM") as tppool,
        ):
            tri_sb = persist.tile([P, P], BF16)
            ident_sb = persist.tile([P, P], BF16)
            bout_sb = persist.tile([P, DIM], F32)

            v_drams = [dram.tile([N, HPC * P], BF16, name=f"v_dram{b}")
                       for b in range(B)]
            a2a_in = {
                'A': dram.tile([W, HPC * P, 4 * P], BF16, name="a2aA_in"),
                'B': dram.tile([W, HPC * P, 4 * P], BF16, name="a2aB_in"),
            }
            a2a_out = {
                'A': dram.tile([W, HPC * P, 4 * P], BF16, name="a2aA_out"),
                'B': dram.tile([W, HPC * P, 4 * P], BF16, name="a2aB_out"),
            }

            woutb_r = wout.rearrange("(kt p) c -> p kt c", p=P)
            qk_bs = [qkpool.tile([P, 4, N], BF16, name=f"qkb{b}")
                     for b in range(B)]
            va_tiles = {}

            def load_va(b, hl):
                va = vapool.tile([P, NJT, P + 1], BF16, tag="va")
                nc.vector.memset(va[:, :, P:P + 1], 1.0)
                v_r = v_drams[b].rearrange("(jt p) d -> p jt d", p=P)
                nc.sync.dma_start(va[:, :, :P], v_r[:, :, hl * P:(hl + 1) * P])
                va_tiles[(b, hl)] = va

            last_exp = [None]  # most recent exp instruction (for load pinning)

            def emit_scores(b, hl, c, pool, tag, alloc_nj=None, dve_c0=False):
                """Compute pt = exp(scores) for chunk c of (b, hl); also
                prefetches va for the later attn@v. alloc_nj lets a smaller
                chunk borrow a larger tag's slot shape."""
                if (b, hl) not in va_tiles:
                    load_va(b, hl)
                nj = PT_NJ[c]
                pt = pool.tile([P, alloc_nj or nj, CHW], BF16, tag=tag)
                qT = qk_bs[b][:, hl]
                kTt = qk_bs[b][:, 2 + hl]
                for J in range(nj):
                    k_off = max(0, J - 4 * c)
                    nn_ = CHW - P * k_off
                    i0 = c * CHW + P * k_off
                    pss = psspool.tile([P, CHW], F32, tag="pss")
                    nc.tensor.matmul(
                        pss[:, :nn_], kTt[:, J * P:(J + 1) * P],
                        qT[:, i0:(c + 1) * CHW], start=True, stop=True)
                    last_exp[0] = nc.scalar.activation(
                        pt[:, J, P * k_off:], pss[:, :nn_],
                        mybir.ActivationFunctionType.Exp)
                    if J >= 4 * c and not (c == 0 and J == 0):
                        # diagonal-tile causal mask on the (idle) gpsimd
                        # engine, freeing DVE cycles
                        nc.gpsimd.tensor_mul(
                            pt[:, J, P * k_off:P * (k_off + 1)],
                            pt[:, J, P * k_off:P * (k_off + 1)], tri_sb[:])
                return pt

            # deferred transpose+write: holds (attn_tile, ph, dest, hl, r0)
            # for the 2 most recent attn@v outputs; flushed two i-tiles later
            # so the TensorE transpose never waits on the vector scale even
            # for short chains. Transposed tiles are staged into a per-chunk
            # [P, 4, P] buffer; when all 4 land, ONE contiguous 128KB DRAM
            # write goes out (stores serialize as DIRECT2D on the sequencer,
            # so fewer/bigger writes keep the a2a triggers prompt).
            pending_fin = []
            last_write = {'A': None, 'B': None}
            fin_bufs = {}  # (ph, dest, hl) -> [tile, count]

            def fin_one(item):
                attn, ph, dest, hl, r0, direct = item
                attT = tppool.tile([P, P], BF16, tag="attT")
                nc.tensor.transpose(attT[:], attn[:], ident_sb[:])
                if direct:
                    # per-tile write: lands right after its copy, skipping
                    # the 4-tile staging + big-write latency (used for the
                    # last BC pair, whose tail gates the a2a-B trigger).
                    # Borrows a finpool slot per tile to avoid a new SBUF tag.
                    dt_t = finpool.tile([P, 4, P], BF16, tag="attnC",
                                        name=f"attnD_{ph}_{dest}_{hl}_{r0}")
                    nc.vector.tensor_copy(dt_t[:, 0], attT[:])
                    last_write[ph] = nc.sync.dma_start(
                        a2a_in[ph][dest, hl * P:(hl + 1) * P, r0:r0 + P],
                        dt_t[:, 0])
                    return
                key = (ph, dest, hl)
                if key not in fin_bufs:
                    ac_t = finpool.tile([P, 4, P], BF16, tag="attnC",
                                        name=f"attnC_{ph}_{dest}_{hl}")
                    fin_bufs[key] = [ac_t, 0]
                ac = fin_bufs[key]
                nc.vector.tensor_copy(ac[0][:, r0 // P], attT[:])
                ac[1] += 1
                if ac[1] == 4:
                    last_write[ph] = nc.sync.dma_start(
                        a2a_in[ph][dest, hl * P:(hl + 1) * P, :],
                        ac[0][:])
                    del fin_bufs[key]

            def flush_fin():
                while pending_fin:
                    fin_one(pending_fin.pop(0))
                assert not fin_bufs, f"incomplete payload chunks: {list(fin_bufs)}"

            def emit_attnv(b, hl, c, pt, pp_list, direct=False):
                """attn@v + normalize for the given pp's of chunk c; the
                transpose+payload-write of each tile is deferred by one."""
                va = va_tiles[(b, hl)]
                for pp in pp_list:
                    it = 4 * c + pp
                    att = attpool.tile([P, P + 1], F32, tag="att")
                    for J in range(it + 1):
                        nc.tensor.matmul(
                            att[:], pt[:, J, P * pp:P * (pp + 1)], va[:, J],
                            start=(J == 0), stop=(J == it))
                    recip = smpool.tile([P, 1], F32, tag="recip")
                    nc.vector.reciprocal(recip[:], att[:, P:P + 1])
                    attn = smpool.tile([P, P], BF16, tag="attn")
                    nc.vector.tensor_scalar_mul(attn[:], att[:, :P], recip[:])
                    dest = b * 2 + it // 8
                    rel = it % 8
                    if rel >= 4:
                        ph, r0 = 'A', (rel - 4) * P
                    else:
                        ph, r0 = 'B', rel * P
                    pending_fin.append((attn, ph, dest, hl, r0, direct))
                    while len(pending_fin) > 2:
                        fin_one(pending_fin.pop(0))

            # ================= scope 1: stage 1 + phase-A attention =========
            pts = {}
            with (
                tc.tile_pool(name="s1w", bufs=1) as s1w,
                tc.tile_pool(name="s1v", bufs=3) as s1v_pool,
                tc.tile_pool(name="s1xf", bufs=2) as s1xf,
                tc.tile_pool(name="s1ps", bufs=2, space="PSUM") as s1ps,
                tc.tile_pool(name="ptA", bufs=2) as ptA,
            ):
                wq_bf = s1w.tile([P, KT, 6 * P], BF16)
                wq_r = wq.rearrange("(kt p) c -> p kt c", p=P)
                xT_r = xT.rearrange("(kt p) n -> p kt n", p=P)
                xb_tiles = {}

                def prefetch_xb(b, c, split):
                    xb = s1xf.tile([P, KT, S1CH], BF16, tag="xb")
                    seq0 = b * N + c * S1CH
                    if split:
                        for kq in range(4):
                            # quarter 0 first on the (otherwise empty) scalar
                            # queue so the kt-split first matmul starts early
                            eng = nc.scalar if kq % 2 == 0 else nc.sync
                            eng.dma_start(
                                xb[:, 4 * kq:4 * (kq + 1)],
                                xT_r[:, 4 * kq:4 * (kq + 1), seq0:seq0 + S1CH])
                    else:
                        nc.sync.dma_start(xb[:], xT_r[:, :, seq0:seq0 + S1CH])
                    xb_tiles[(b, c)] = xb

                def emit_s1_chunk(b, c):
                    # prefetch next chunk's x before computing this one
                    nxt = b * NBC + c + 1
                    if nxt < B * NBC:
                        prefetch_xb(nxt // NBC, nxt % NBC, split=False)
                    xb = xb_tiles.pop((b, c))
                    if (b, c) == (0, 0):
                        # kt-quartered accumulation: the first matmul needs
                        # only the first wq/xb quarter (1.25MB), not the full
                        # 5MB — PE starts ~5us earlier
                        for mg in range(2):
                            psp = [s1ps.tile([P, CHW], F32, tag="ps1",
                                             name=f"ps1q{mg}{i}")
                                   for i in range(2)]
                            for ktq in range(4):
                                for mi in range(2):
                                    for kt in range(4 * ktq, 4 * ktq + 4):
                                        nc.tensor.matmul(
                                            psp[mi][:, :S1CH],
                                            wq_bf[:, kt,
                                                  (2 * mg + mi) * P:
                                                  (2 * mg + mi + 1) * P],
                                            xb[:, kt],
                                            start=(kt == 0),
                                            stop=(kt == KT - 1))
                            for mi in range(2):
                                nc.vector.tensor_copy(
                                    qk_bs[b][:, 2 * mg + mi, :S1CH],
                                    psp[mi][:, :S1CH])
                    else:
                        for m in range(4):
                            ps = s1ps.tile([P, CHW], F32, tag="ps1")
                            for kt in range(KT):
                                nc.tensor.matmul(
                                    ps[:, :S1CH],
                                    wq_bf[:, kt, m * P:(m + 1) * P],
                                    xb[:, kt],
                                    start=(kt == 0), stop=(kt == KT - 1))
                            nc.vector.tensor_copy(
                                qk_bs[b][:, m, c * S1CH:(c + 1) * S1CH],
                                ps[:, :S1CH])
                    for st2 in range(S1CH // P):
                        st = c * (S1CH // P) + st2
                        psv = s1ps.tile([P, CHW], F32, tag="ps1")
                        for kt in range(KT):
                            nc.tensor.matmul(
                                psv[:, :HPC * P], xb[:, kt, st2 * P:(st2 + 1) * P],
                                wq_bf[:, kt, 4 * P:6 * P],
                                start=(kt == 0), stop=(kt == KT - 1))
                        vst = s1v_pool.tile([P, HPC * P], BF16, tag="vst")
                        nc.vector.tensor_copy(vst[:], psv[:, :HPC * P])
                        nc.sync.dma_start(
                            v_drams[b][st * P:(st + 1) * P, :], vst[:])

                # startup loads, interleaved across both HWDGE queues in the
                # exact order the kt-quartered first chunk consumes them:
                # ktq0 needs wq[0:4] (sync) + xb q0 (scalar) = 1.25MB only
                xb00 = s1xf.tile([P, KT, S1CH], BF16, tag="xb")
                nc.sync.dma_start(wq_bf[:, 0:4], wq_r[:, 0:4])
                nc.scalar.dma_start(xb00[:, 0:4], xT_r[:, 0:4, :S1CH])
                nc.sync.dma_start(xb00[:, 4:8], xT_r[:, 4:8, :S1CH])
                nc.scalar.dma_start(wq_bf[:, 4:8], wq_r[:, 4:8])
                nc.sync.dma_start(wq_bf[:, 8:12], wq_r[:, 8:12])
                nc.scalar.dma_start(xb00[:, 8:12], xT_r[:, 8:12, :S1CH])
                nc.sync.dma_start(xb00[:, 12:16], xT_r[:, 12:16, :S1CH])
                nc.scalar.dma_start(wq_bf[:, 12:16], wq_r[:, 12:16])
                xb_tiles[(0, 0)] = xb00
                # mask/identity tiles: small loads queued behind the first
                # chunk (first needed at batch-0 scores, ~130us in)
                nc.sync.dma_start(tri_sb[:], tri.ap())
                nc.scalar.dma_start(ident_sb[:], ident.ap())
                for b in range(B):
                    for c in range(NBC):
                        emit_s1_chunk(b, c)
                        if b >= 1:
                            pb = b - 1
                            if c == 0:
                                emit_attnv(pb, 0, 1, pts[(pb, 0, 1)], [0, 1, 2, 3])
                            elif c == 1:
                                emit_attnv(pb, 0, 3, pts[(pb, 0, 3)], [0, 1, 2, 3])
                                del va_tiles[(pb, 0)]  # phase-A use done
                            elif c == 2:
                                pts[(pb, 1, 1)] = emit_scores(pb, 1, 1, ptA, "ptc1")
                                pts[(pb, 1, 3)] = emit_scores(pb, 1, 3, ptA, "ptc3")
                            else:
                                emit_attnv(pb, 1, 1, pts[(pb, 1, 1)], [0, 1, 2, 3])
                                emit_attnv(pb, 1, 3, pts[(pb, 1, 3)], [0, 1, 2, 3])
                                del va_tiles[(pb, 1)]  # phase-A use done
                    pts[(b, 0, 1)] = emit_scores(b, 0, 1, ptA, "ptc1")
                    pts[(b, 0, 3)] = emit_scores(b, 0, 3, ptA, "ptc3")

                # tail: batch 3 phase-A attention (no s1 left to hide under)
                b = B - 1
                pts[(b, 1, 1)] = emit_scores(b, 1, 1, ptA, "ptc1")
                emit_attnv(b, 0, 1, pts[(b, 0, 1)], [0, 1, 2, 3])
                pts[(b, 1, 3)] = emit_scores(b, 1, 3, ptA, "ptc3")
                emit_attnv(b, 0, 3, pts[(b, 0, 3)], [0, 1, 2, 3])
                emit_attnv(b, 1, 1, pts[(b, 1, 1)], [0, 1, 2, 3])
                emit_attnv(b, 1, 3, pts[(b, 1, 3)], [0, 1, 2, 3])
                flush_fin()

            nc.gpsimd.collective_compute(
                "AllToAll", mybir.AluOpType.bypass,
                replica_groups=[list(range(W))],
                ins=[a2a_in['A'][:].opt()], outs=[a2a_out['A'][:].opt()],
            )

            # ============== scope 2: BC attention + stage 4 =================
            with (
                tc.tile_pool(name="ptBC", bufs=2) as ptBC,
                tc.tile_pool(name="s4l", bufs=1) as s4l,
                tc.tile_pool(name="s4w", bufs=4) as s4w,
                tc.tile_pool(name="s4o", bufs=8) as s4o,
                tc.tile_pool(name="s4ps", bufs=2, space="PSUM") as s4ps,
            ):
                lhsA = s4l.tile([P, KT, 4 * P], BF16, name="lhsA")
                lhsB = s4l.tile([P, KT, 4 * P], BF16, name="lhsB")

                def lhs_load(dst, ph, nsplit):
                    # plain chunked DMA on the scalar (Activation) DGE queue:
                    # pinned after the last exp so the static scheduler cannot
                    # hoist it ahead of pending exps (its collective wait
                    # would stall them); waits there blocking nothing.
                    # Split along the m/row dim so the first s4 block of the
                    # pass can start after the first quarter arrives.
                    view = (a2a_out[ph].rearrange("w h r -> (w h) r")
                            .rearrange("(kt p) r -> p kt r", p=P))
                    step = (4 * P) // nsplit
                    for q in range(nsplit):
                        d = nc.scalar.dma_start(
                            dst[:, :, q * step:(q + 1) * step],
                            view[:, :, q * step:(q + 1) * step])
                        add_dep_helper(d.ins, last_exp[0].ins, sync=False,
                                       reason="lhs load after all exps")

                S4C = 256          # stage-4 column chunk
                NS4 = DIM // S4C   # 8 chunks

                def load_wb(ncx, pin):
                    # wout chunk load, split in two; optionally pinned after
                    # a payload write so the scheduler can't hoist it into
                    # the BC stream's DMA window
                    wb = s4w.tile([P, KT, S4C], BF16, tag="wb")
                    for h in range(2):
                        d = nc.sync.dma_start(
                            wb[:, 8 * h:8 * (h + 1)],
                            woutb_r[:, 8 * h:8 * (h + 1),
                                    ncx * S4C:(ncx + 1) * S4C])
                        if pin is not None:
                            add_dep_helper(d.ins, pin.ins, sync=False,
                                           reason="wb after attn writes")
                    return wb

                def emit_s4_block(lhs, m0, lc0, nm, ncx, wb, out_eng=None):
                    for ml in range(nm):
                        m = m0 + ml
                        lc = lc0 + ml
                        ps4 = s4ps.tile([P, S4C], F32, tag="ps4")
                        for kt in range(KT):
                            nc.tensor.matmul(
                                ps4[:], lhs[:, kt, lc * P:(lc + 1) * P],
                                wb[:, kt],
                                start=(kt == 0), stop=(kt == KT - 1))
                        osb = s4o.tile([P, S4C], F32, tag="osb")
                        nc.vector.tensor_tensor(
                            osb[:], ps4[:],
                            bout_sb[:, ncx * S4C:(ncx + 1) * S4C],
                            mybir.AluOpType.add)
                        # alternate sequencers: DIRECT2D stores run at
                        # ~56GB/s serialized per queue; one queue alone
                        # cannot keep up with stage-4's output rate
                        eng = out_eng or (
                            nc.sync if (m + ncx) % 2 == 0 else nc.scalar)
                        eng.dma_start(
                            out[m * P:(m + 1) * P,
                                ncx * S4C:(ncx + 1) * S4C],
                            osb[:])

                # bias + out-proj weight chunks 0-3 preloaded early (no deps)
                nc.sync.dma_start(
                    bout_sb[:], bout.ap().to_broadcast((P, DIM)))
                wbs = {n: load_wb(n, None) for n in range(4)}

                # ALL phase-B (BC) attention runs here, hiding a2a-A
                pairs = [(b, hl) for b in range(B) for hl in range(HPC)]
                va_tiles.clear()  # phase-A va slots are stale; reload per pair

                def emit_bc_scores(k):
                    b, hl = pairs[k]
                    pts[(b, hl, 0)] = emit_scores(b, hl, 0, ptBC, "ptc0",
                                                  dve_c0=True)
                    pts[(b, hl, 2)] = emit_scores(b, hl, 2, ptBC, "ptc2")

                emit_bc_scores(0)
                emit_bc_scores(1)
                last_k = len(pairs) - 1
                for k, (b, hl) in enumerate(pairs):
                    # big chunk (c2) first, tiny c0 chain last: the slowest
                    # core's a2a-B trigger waits on the tail of this chain,
                    # so end each pair with the shortest dependency tail.
                    # The last pair writes per-tile (not chunk-batched) so
                    # its payload lands with minimal staging latency.
                    d = (k == last_k)
                    emit_attnv(b, hl, 2, pts[(b, hl, 2)], [2, 3], direct=d)
                    emit_attnv(b, hl, 0, pts[(b, hl, 0)], [2, 3], direct=d)
                    emit_attnv(b, hl, 2, pts[(b, hl, 2)], [0, 1], direct=d)
                    emit_attnv(b, hl, 0, pts[(b, hl, 0)], [0, 1], direct=d)
                    if k + 2 < len(pairs):
                        emit_bc_scores(k + 2)

                flush_fin()
                # collectives first: their triggers drain program-order-prior
                # DMA work, so nothing else may be emitted before them
                nc.gpsimd.collective_compute(
                    "AllToAll", mybir.AluOpType.bypass,
                    replica_groups=[list(range(W))],
                    ins=[a2a_in['B'][:].opt()], outs=[a2a_out['B'][:].opt()],
                )

                lhs_load(lhsA, 'A', 4)

                def s4_pass(lhs, m0, nm, wbp, ncx0=0):
                    for ncx in range(ncx0, NS4):
                        if ncx + 3 < NS4:
                            wbp[ncx + 3] = load_wb(ncx + 3, last_write['B'])
                        emit_s4_block(lhs, m0, 0, nm, ncx, wbp.pop(ncx))

                s4_pass(lhsA, 4, 4, wbs)
                # lhsB loads emitted only now: they wait on a2a-B completion,
                # and anything queued behind them on the scalar DGE queue
                # (half the osb stores) would wedge with them if emitted
                # before pass A.
                lhs_load(lhsB, 'B', 4)
                s4_pass(lhsB, 0, 4, {0: load_wb(0, last_write['B']),
                                     1: load_wb(1, last_write['B']),
                                     2: load_wb(2, last_write['B'])})

    nc.compile()
    return nc


_NC = None


def _get_nc():
    global _NC
    if _NC is None:
        _NC = build()
    return _NC


last_exec_time_ns = None
last_results = None


def kernel(x, w_qkv, w_out, b_out):
    global last_exec_time_ns, last_results
    _install_ntff_hook()
    nc = _get_nc()

    x = np.asarray(x, dtype=np.float32)
    w_qkv = np.asarray(w_qkv, dtype=np.float32)
    w_out = np.asarray(w_out, dtype=np.float32)
    b_out = np.asarray(b_out, dtype=np.float32)

    bf = ml_dtypes.bfloat16
    xT = np.ascontiguousarray(x.reshape(ROWS, DIM).T.astype(bf))
    wout_b = np.ascontiguousarray(w_out.astype(bf))
    bout2 = np.ascontiguousarray(b_out.reshape(1, DIM))

    in_maps = []
    for core in range(W):
        cols = [w_qkv[:, part * (HEADS * D) + core * HPC * D:
                      part * (HEADS * D) + (core + 1) * HPC * D]
                for part in range(3)]
        cols[0] = cols[0] * SCALE  # fold softmax scale into q projection
        wq_c = np.ascontiguousarray(np.concatenate(cols, axis=1).astype(bf))
        in_maps.append({"xT": xT, "wq": wq_c, "wout": wout_b, "bout": bout2})

    trace = os.environ.get("KERNEL_TRACE", "") not in ("", "0")
    res = bass_utils.run_bass_kernel_spmd(
        nc, in_maps, core_ids=list(range(W)), trace=trace)
    last_exec_time_ns = res.exec_time_ns
    last_results = res

    out = np.concatenate([res.results[c]["out"] for c in range(W)], axis=0)
    return np.ascontiguousarray(out.reshape(B, N, DIM), dtype=np.float32)



# revision 50
# speedup vs baseline: 1.0008x; 1.0008x over previous
"""Trainium2 Bass kernel for nn_Attention_23476291240422 (sparse attention:
causal + 128-wide noncausal prefix block; b=4, n=2048, dim=2048, 16 heads,
d=128) distributed across 8 NeuronCores.

v4e design for gapless TensorE (PE sustains ~1.9 GHz; the kernel is row-count
bound at ~1.33M matmul rows/core, so every PE idle ns is pure loss):
- [inner, rows] AllToAll payloads; the [i,d]->[d,i] flip runs on TensorE,
  one transpose per i-tile, staged into per-chunk [P,4,P] buffers so each
  payload store is ONE contiguous 128KB write (SBUF->DRAM stores serialize
  as DIRECT2D on the issuing sequencer at ~56GB/s — write count matters).
- 2 collective phases by query-position: phase A = query chunks 1,3, phase
  B = chunks 0,2. Scope 1 computes stage-1 + ALL phase-A attention, then
  a2a-A fires and hides under scope 2's phase-B (BC) attention. a2a-B then
  hides under stage-4 pass A (lhsA only needs a2a-A). lhs loads are chunked
  DMAs on the scalar (Activation) DGE queue, pinned after the last exp;
  lhsB's loads are emitted only after pass A so nothing queues behind their
  collective wait. Collectives have a ~20us bulk phase that monopolizes all
  16 DMA engines: everything overlapping a collective is preloaded (va
  bufs=5, wb prefetch distance 3) or buffered (s4o bufs=8, fin bufs=6).
- Startup: kt-quartered accumulation for the first s1 chunk — the first
  matmul needs only 1.25MB (first wq+xb quarters), loads interleaved across
  both HWDGE queues in consumption order; bout load deferred to scope 2.
- Stage-4 in 256-col chunks, output stores alternating sync/scalar
  sequencers (one sequencer alone cannot keep up with stage-4's rate).
- Deep software pipelining: phase-A attention of batch b interleaved inside
  stage-1 of batch b+1; scores of the next (batch,head) pair emitted before
  attn@v of the current one so scalar-engine exp latency is hidden; xb
  prefetched one chunk ahead; causal diagonal masks on the idle gpsimd.
- Softmax scale folded into the q columns of w_qkv on the host.
"""
import os
import sys
import types

import numpy as np
import ml_dtypes

import concourse.bass as bass
import concourse.mybir as mybir
import concourse.tile as tile
from concourse.tile import add_dep_helper
from concourse import bacc
from concourse import bass_utils

B, N, DIM = 4, 2048, 2048
HEADS, D, L = 16, 128, 128
W = 8
HPC = HEADS // W          # 2 heads per core
ROWS = B * N              # 8192
RPC = ROWS // W           # 1024 rows per core
SCALE = float(D) ** -0.5
P = 128
KT = DIM // P             # 16
S1CH = 512                # stage-1 seq chunk width
NBC = N // S1CH           # 4 stage-1 chunks per batch
CHW = 512                 # score chunk width
NJT = N // P              # 16 j-tiles
F32, BF16 = mybir.dt.float32, mybir.dt.bfloat16

# attention chunks (512 queries each): phase A = full chunks 1,3
# (it%8 in 4..7 -> dest rows 512-1023); chunks 0,2 split by pp:
# pp 2,3 -> phase B (rows 256-511), pp 0,1 -> phase C (rows 0-255).
PT_NJ = {0: 4, 1: 8, 2: 12, 3: 16}   # j-tiles needed per chunk


def _install_ntff_hook():
    try:
        import antenv.axon_hooks  # noqa: F401
        return
    except ImportError:
        pass
    try:
        import antenv
        from trn_agent_boot.trn_boot import _ntff_profile_via_ctypes
        hook = [_ntff_profile_via_ctypes("/opt/axon/libaxon_pjrt.so")]
        mod = types.ModuleType("antenv.axon_hooks")
        mod.get_axon_ntff_profile_hook = lambda: hook[0]
        mod.set_axon_ntff_profile_hook = lambda h: hook.__setitem__(0, h)
        sys.modules["antenv.axon_hooks"] = mod
        antenv.axon_hooks = mod
    except Exception:
        pass


def build():
    nc = bacc.Bacc("TRN2", target_bir_lowering=False, debug=False, num_devices=W)

    xT = nc.dram_tensor("xT", [DIM, ROWS], BF16, kind="ExternalInput")
    wq = nc.dram_tensor("wq", [DIM, 6 * P], BF16, kind="ExternalInput")  # q0 q1 k0 k1 v0 v1 (q pre-scaled)
    wout = nc.dram_tensor("wout", [DIM, DIM], BF16, kind="ExternalInput")
    bout = nc.dram_tensor("bout", [1, DIM], F32, kind="ExternalInput")
    out = nc.dram_tensor("out", [RPC, DIM], F32, kind="ExternalOutput")

    tri_np = (np.arange(P)[:, None] <= np.arange(P)[None, :]).astype(ml_dtypes.bfloat16)
    tri = nc.inline_tensor(tri_np, name="tri")
    ident = nc.inline_tensor(np.eye(P, dtype=ml_dtypes.bfloat16), name="ident")

    with tile.TileContext(nc) as tc:
        with (
            tc.tile_pool(name="persist", bufs=1) as persist,
            tc.tile_pool(name="dram", bufs=1, space="DRAM") as dram,
            tc.tile_pool(name="qk", bufs=1) as qkpool,
            tc.tile_pool(name="va", bufs=5) as vapool,
            tc.tile_pool(name="sm", bufs=8) as smpool,
            tc.tile_pool(name="fin", bufs=6) as finpool,
            tc.tile_pool(name="pss", bufs=2, space="PSUM") as psspool,
            tc.tile_pool(name="att", bufs=2, space="PSUM") as attpool,
            tc.tile_pool(name="s2tp", bufs=2, space="PSUM") as tppool,
        ):
            tri_sb = persist.tile([P, P], BF16)
            ident_sb = persist.tile([P, P], BF16)
            bout_sb = persist.tile([P, DIM], F32)

            v_drams = [dram.tile([N, HPC * P], BF16, name=f"v_dram{b}")
                       for b in range(B)]
            a2a_in = {
                'A': dram.tile([W, HPC * P, 4 * P], BF16, name="a2aA_in"),
                'B': dram.tile([W, HPC * P, 4 * P], BF16, name="a2aB_in"),
            }
            a2a_out = {
                'A': dram.tile([W, HPC * P, 4 * P], BF16, name="a2aA_out"),
                'B': dram.tile([W, HPC * P, 4 * P], BF16, name="a2aB_out"),
            }

            woutb_r = wout.rearrange("(kt p) c -> p kt c", p=P)
            qk_bs = [qkpool.tile([P, 4, N], BF16, name=f"qkb{b}")
                     for b in range(B)]
            va_tiles = {}

            def load_va(b, hl):
                va = vapool.tile([P, NJT, P + 1], BF16, tag="va")
                nc.vector.memset(va[:, :, P:P + 1], 1.0)
                v_r = v_drams[b].rearrange("(jt p) d -> p jt d", p=P)
                nc.sync.dma_start(va[:, :, :P], v_r[:, :, hl * P:(hl + 1) * P])
                va_tiles[(b, hl)] = va

            last_exp = [None]  # most recent exp instruction (for load pinning)

            def emit_scores(b, hl, c, pool, tag, alloc_nj=None, dve_c0=False):
                """Compute pt = exp(scores) for chunk c of (b, hl); also
                prefetches va for the later attn@v. alloc_nj lets a smaller
                chunk borrow a larger tag's slot shape."""
                if (b, hl) not in va_tiles:
                    load_va(b, hl)
                nj = PT_NJ[c]
                pt = pool.tile([P, alloc_nj or nj, CHW], BF16, tag=tag)
                qT = qk_bs[b][:, hl]
                kTt = qk_bs[b][:, 2 + hl]
                for J in range(nj):
                    k_off = max(0, J - 4 * c)
                    nn_ = CHW - P * k_off
                    i0 = c * CHW + P * k_off
                    pss = psspool.tile([P, CHW], F32, tag="pss")
                    nc.tensor.matmul(
                        pss[:, :nn_], kTt[:, J * P:(J + 1) * P],
                        qT[:, i0:(c + 1) * CHW], start=True, stop=True)
                    last_exp[0] = nc.scalar.activation(
                        pt[:, J, P * k_off:], pss[:, :nn_],
                        mybir.ActivationFunctionType.Exp)
                    if J >= 4 * c and not (c == 0 and J == 0):
                        # diagonal-tile causal mask on the (idle) gpsimd
                        # engine, freeing DVE cycles
                        nc.gpsimd.tensor_mul(
                            pt[:, J, P * k_off:P * (k_off + 1)],
                            pt[:, J, P * k_off:P * (k_off + 1)], tri_sb[:])
                return pt

            # deferred transpose+write: holds (attn_tile, ph, dest, hl, r0)
            # for the 2 most recent attn@v outputs; flushed two i-tiles later
            # so the TensorE transpose never waits on the vector scale even
            # for short chains. Transposed tiles are staged into a per-chunk
            # [P, 4, P] buffer; when all 4 land, ONE contiguous 128KB DRAM
            # write goes out (stores serialize as DIRECT2D on the sequencer,
            # so fewer/bigger writes keep the a2a triggers prompt).
            pending_fin = []
            last_write = {'A': None, 'B': None}
            fin_bufs = {}  # (ph, dest, hl) -> [tile, count]

            def fin_one(item):
                attn, ph, dest, hl, r0, direct = item
                attT = tppool.tile([P, P], BF16, tag="attT")
                nc.tensor.transpose(attT[:], attn[:], ident_sb[:])
                if direct:
                    # per-tile write: lands right after its copy, skipping
                    # the 4-tile staging + big-write latency (used for the
                    # last BC pair, whose tail gates the a2a-B trigger).
                    # Borrows a finpool slot per tile to avoid a new SBUF tag.
                    dt_t = finpool.tile([P, 4, P], BF16, tag="attnC",
                                        name=f"attnD_{ph}_{dest}_{hl}_{r0}")
                    nc.vector.tensor_copy(dt_t[:, 0], attT[:])
                    last_write[ph] = nc.sync.dma_start(
                        a2a_in[ph][dest, hl * P:(hl + 1) * P, r0:r0 + P],
                        dt_t[:, 0])
                    return
                key = (ph, dest, hl)
                if key not in fin_bufs:
                    ac_t = finpool.tile([P, 4, P], BF16, tag="attnC",
                                        name=f"attnC_{ph}_{dest}_{hl}")
                    fin_bufs[key] = [ac_t, 0]
                ac = fin_bufs[key]
                nc.vector.tensor_copy(ac[0][:, r0 // P], attT[:])
                ac[1] += 1
                if ac[1] == 4:
                    last_write[ph] = nc.sync.dma_start(
                        a2a_in[ph][dest, hl * P:(hl + 1) * P, :],
                        ac[0][:])
                    del fin_bufs[key]

            def flush_fin():
                while pending_fin:
                    fin_one(pending_fin.pop(0))
                assert not fin_bufs, f"incomplete payload chunks: {list(fin_bufs)}"

            def emit_attnv(b, hl, c, pt, pp_list, direct=False):
                """attn@v + normalize for the given pp's of chunk c; the
                transpose+payload-write of each tile is deferred by one."""
                va = va_tiles[(b, hl)]
                for pp in pp_list:
                    it = 4 * c + pp
                    att = attpool.tile([P, P + 1], F32, tag="att")
                    for J in range(it + 1):
                        nc.tensor.matmul(
                            att[:], pt[:, J, P * pp:P * (pp + 1)], va[:, J],
                            start=(J == 0), stop=(J == it))
                    recip = smpool.tile([P, 1], F32, tag="recip")
                    nc.vector.reciprocal(recip[:], att[:, P:P + 1])
                    attn = smpool.tile([P, P], BF16, tag="attn")
                    nc.vector.tensor_scalar_mul(attn[:], att[:, :P], recip[:])
                    dest = b * 2 + it // 8
                    rel = it % 8
                    if rel >= 4:
                        ph, r0 = 'A', (rel - 4) * P
                    else:
                        ph, r0 = 'B', rel * P
                    pending_fin.append((attn, ph, dest, hl, r0, direct))
                    while len(pending_fin) > 2:
                        fin_one(pending_fin.pop(0))

            # ================= scope 1: stage 1 + phase-A attention =========
            pts = {}
            with (
                tc.tile_pool(name="s1w", bufs=1) as s1w,
                tc.tile_pool(name="s1v", bufs=3) as s1v_pool,
                tc.tile_pool(name="s1xf", bufs=2) as s1xf,
                tc.tile_pool(name="s1ps", bufs=2, space="PSUM") as s1ps,
                tc.tile_pool(name="ptA", bufs=2) as ptA,
            ):
                wq_bf = s1w.tile([P, KT, 6 * P], BF16)
                wq_r = wq.rearrange("(kt p) c -> p kt c", p=P)
                xT_r = xT.rearrange("(kt p) n -> p kt n", p=P)
                xb_tiles = {}

                def prefetch_xb(b, c, split):
                    xb = s1xf.tile([P, KT, S1CH], BF16, tag="xb")
                    seq0 = b * N + c * S1CH
                    if split:
                        for kq in range(4):
                            # quarter 0 first on the (otherwise empty) scalar
                            # queue so the kt-split first matmul starts early
                            eng = nc.scalar if kq % 2 == 0 else nc.sync
                            eng.dma_start(
                                xb[:, 4 * kq:4 * (kq + 1)],
                                xT_r[:, 4 * kq:4 * (kq + 1), seq0:seq0 + S1CH])
                    else:
                        nc.sync.dma_start(xb[:], xT_r[:, :, seq0:seq0 + S1CH])
                    xb_tiles[(b, c)] = xb

                def emit_s1_chunk(b, c):
                    # prefetch next chunk's x before computing this one
                    nxt = b * NBC + c + 1
                    if nxt < B * NBC:
                        prefetch_xb(nxt // NBC, nxt % NBC, split=False)
                    xb = xb_tiles.pop((b, c))
                    if (b, c) == (0, 0):
                        # kt-quartered accumulation: the first matmul needs
                        # only the first wq/xb quarter (1.25MB), not the full
                        # 5MB — PE starts ~5us earlier
                        for mg in range(2):
                            psp = [s1ps.tile([P, CHW], F32, tag="ps1",
                                             name=f"ps1q{mg}{i}")
                                   for i in range(2)]
                            for ktq in range(4):
                                for mi in range(2):
                                    for kt in range(4 * ktq, 4 * ktq + 4):
                                        nc.tensor.matmul(
                                            psp[mi][:, :S1CH],
                                            wq_bf[:, kt,
                                                  (2 * mg + mi) * P:
                                                  (2 * mg + mi + 1) * P],
                                            xb[:, kt],
                                            start=(kt == 0),
                                            stop=(kt == KT - 1))
                            for mi in range(2):
                                nc.vector.tensor_copy(
                                    qk_bs[b][:, 2 * mg + mi, :S1CH],
                                    psp[mi][:, :S1CH])
                    else:
                        for m in range(4):
                            ps = s1ps.tile([P, CHW], F32, tag="ps1")
                            for kt in range(KT):
                                nc.tensor.matmul(
                                    ps[:, :S1CH],
                                    wq_bf[:, kt, m * P:(m + 1) * P],
                                    xb[:, kt],
                                    start=(kt == 0), stop=(kt == KT - 1))
                            nc.vector.tensor_copy(
                                qk_bs[b][:, m, c * S1CH:(c + 1) * S1CH],
                                ps[:, :S1CH])
                    for st2 in range(S1CH // P):
                        st = c * (S1CH // P) + st2
                        psv = s1ps.tile([P, CHW], F32, tag="ps1")
                        for kt in range(KT):
                            nc.tensor.matmul(
                                psv[:, :HPC * P], xb[:, kt, st2 * P:(st2 + 1) * P],
                                wq_bf[:, kt, 4 * P:6 * P],
                                start=(kt == 0), stop=(kt == KT - 1))
                        vst = s1v_pool.tile([P, HPC * P], BF16, tag="vst")
                        nc.vector.tensor_copy(vst[:], psv[:, :HPC * P])
                        nc.sync.dma_start(
                            v_drams[b][st * P:(st + 1) * P, :], vst[:])

                # startup loads, interleaved across both HWDGE queues in the
                # exact order the kt-quartered first chunk consumes them:
                # ktq0 needs wq[0:4] (sync) + xb q0 (scalar) = 1.25MB only
                xb00 = s1xf.tile([P, KT, S1CH], BF16, tag="xb")
                nc.sync.dma_start(wq_bf[:, 0:4], wq_r[:, 0:4])
                nc.scalar.dma_start(xb00[:, 0:4], xT_r[:, 0:4, :S1CH])
                nc.sync.dma_start(xb00[:, 4:8], xT_r[:, 4:8, :S1CH])
                nc.scalar.dma_start(wq_bf[:, 4:8], wq_r[:, 4:8])
                nc.sync.dma_start(wq_bf[:, 8:12], wq_r[:, 8:12])
                nc.scalar.dma_start(xb00[:, 8:12], xT_r[:, 8:12, :S1CH])
                nc.sync.dma_start(xb00[:, 12:16], xT_r[:, 12:16, :S1CH])
                nc.scalar.dma_start(wq_bf[:, 12:16], wq_r[:, 12:16])
                xb_tiles[(0, 0)] = xb00
                # mask/identity tiles: small loads queued behind the first
                # chunk (first needed at batch-0 scores, ~130us in)
                nc.sync.dma_start(tri_sb[:], tri.ap())
                nc.scalar.dma_start(ident_sb[:], ident.ap())
                for b in range(B):
                    for c in range(NBC):
                        emit_s1_chunk(b, c)
                        if b >= 1:
                            pb = b - 1
                            if c == 0:
                                emit_attnv(pb, 0, 1, pts[(pb, 0, 1)], [0, 1, 2, 3])
                            elif c == 1:
                                emit_attnv(pb, 0, 3, pts[(pb, 0, 3)], [0, 1, 2, 3])
                                del va_tiles[(pb, 0)]  # phase-A use done
                            elif c == 2:
                                pts[(pb, 1, 1)] = emit_scores(pb, 1, 1, ptA, "ptc1")
                                pts[(pb, 1, 3)] = emit_scores(pb, 1, 3, ptA, "ptc3")
                            else:
                                emit_attnv(pb, 1, 1, pts[(pb, 1, 1)], [0, 1, 2, 3])
                                emit_attnv(pb, 1, 3, pts[(pb, 1, 3)], [0, 1, 2, 3])
                                del va_tiles[(pb, 1)]  # phase-A use done
                    pts[(b, 0, 1)] = emit_scores(b, 0, 1, ptA, "ptc1")
                    pts[(b, 0, 3)] = emit_scores(b, 0, 3, ptA, "ptc3")

                # tail: batch 3 phase-A attention (no s1 left to hide under)
                b = B - 1
                pts[(b, 1, 1)] = emit_scores(b, 1, 1, ptA, "ptc1")
                emit_attnv(b, 0, 1, pts[(b, 0, 1)], [0, 1, 2, 3])
                pts[(b, 1, 3)] = emit_scores(b, 1, 3, ptA, "ptc3")
                emit_attnv(b, 0, 3, pts[(b, 0, 3)], [0, 1, 2, 3])
                emit_attnv(b, 1, 1, pts[(b, 1, 1)], [0, 1, 2, 3])
                emit_attnv(b, 1, 3, pts[(b, 1, 3)], [0, 1, 2, 3])
                flush_fin()

            nc.gpsimd.collective_compute(
                "AllToAll", mybir.AluOpType.bypass,
                replica_groups=[list(range(W))],
                ins=[a2a_in['A'][:].opt()], outs=[a2a_out['A'][:].opt()],
            )

            # ============== scope 2: BC attention + stage 4 =================
            with (
                tc.tile_pool(name="ptBC", bufs=2) as ptBC,
                tc.tile_pool(name="s4l", bufs=1) as s4l,
                tc.tile_pool(name="s4w", bufs=4) as s4w,
                tc.tile_pool(name="s4o", bufs=8) as s4o,
                tc.tile_pool(name="s4ps", bufs=2, space="PSUM") as s4ps,
            ):
                lhsA = s4l.tile([P, KT, 4 * P], BF16, name="lhsA")
                lhsB = s4l.tile([P, KT, 4 * P], BF16, name="lhsB")

                def lhs_load(dst, ph, nsplit):
                    # plain chunked DMA on the scalar (Activation) DGE queue:
                    # pinned after the last exp so the static scheduler cannot
                    # hoist it ahead of pending exps (its collective wait
                    # would stall them); waits there blocking nothing.
                    # Split along the m/row dim so the first s4 block of the
                    # pass can start after the first quarter arrives.
                    view = (a2a_out[ph].rearrange("w h r -> (w h) r")
                            .rearrange("(kt p) r -> p kt r", p=P))
                    step = (4 * P) // nsplit
                    for q in range(nsplit):
                        d = nc.scalar.dma_start(
                            dst[:, :, q * step:(q + 1) * step],
                            view[:, :, q * step:(q + 1) * step])
                        add_dep_helper(d.ins, last_exp[0].ins, sync=False,
                                       reason="lhs load after all exps")

                S4C = 256          # stage-4 column chunk
                NS4 = DIM // S4C   # 8 chunks

                def load_wb(ncx, pin):
                    # wout chunk load, split in two; optionally pinned after
                    # a payload write so the scheduler can't hoist it into
                    # the BC stream's DMA window
                    wb = s4w.tile([P, KT, S4C], BF16, tag="wb")
                    for h in range(2):
                        d = nc.sync.dma_start(
                            wb[:, 8 * h:8 * (h + 1)],
                            woutb_r[:, 8 * h:8 * (h + 1),
                                    ncx * S4C:(ncx + 1) * S4C])
                        if pin is not None:
                            add_dep_helper(d.ins, pin.ins, sync=False,
                                           reason="wb after attn writes")
                    return wb

                def emit_s4_block(lhs, m0, lc0, nm, ncx, wb, out_eng=None):
                    for ml in range(nm):
                        m = m0 + ml
                        lc = lc0 + ml
                        ps4 = s4ps.tile([P, S4C], F32, tag="ps4")
                        for kt in range(KT):
                            nc.tensor.matmul(
                                ps4[:], lhs[:, kt, lc * P:(lc + 1) * P],
                                wb[:, kt],
                                start=(kt == 0), stop=(kt == KT - 1))
                        osb = s4o.tile([P, S4C], F32, tag="osb")
                        nc.vector.tensor_tensor(
                            osb[:], ps4[:],
                            bout_sb[:, ncx * S4C:(ncx + 1) * S4C],
                            mybir.AluOpType.add)
                        # alternate sequencers: DIRECT2D stores run at
                        # ~56GB/s serialized per queue; one queue alone
                        # cannot keep up with stage-4's output rate
                        eng = out_eng or (
                            nc.sync if (m + ncx) % 2 == 0 else nc.scalar)
                        eng.dma_start(
                            out[m * P:(m + 1) * P,
                                ncx * S4C:(ncx + 1) * S4C],
                            osb[:])

                # bias + out-proj weight chunks 0-3 preloaded early (no deps)
                nc.sync.dma_start(
                    bout_sb[:], bout.ap().to_broadcast((P, DIM)))
                wbs = {n: load_wb(n, None) for n in range(4)}

                # ALL phase-B (BC) attention runs here, hiding a2a-A
                pairs = [(b, hl) for b in range(B) for hl in range(HPC)]
                va_tiles.clear()  # phase-A va slots are stale; reload per pair

                def emit_bc_scores(k):
                    b, hl = pairs[k]
                    pts[(b, hl, 0)] = emit_scores(b, hl, 0, ptBC, "ptc0",
                                                  dve_c0=True)
                    pts[(b, hl, 2)] = emit_scores(b, hl, 2, ptBC, "ptc2")

                emit_bc_scores(0)
                emit_bc_scores(1)
                last_k = len(pairs) - 1
                for k, (b, hl) in enumerate(pairs):
                    # big chunk (c2) first, tiny c0 chain last: the slowest
                    # core's a2a-B trigger waits on the tail of this chain,
                    # so end each pair with the shortest dependency tail.
                    # The last pair writes per-tile (not chunk-batched) so
                    # its payload lands with minimal staging latency.
                    d = (k == last_k)
                    emit_attnv(b, hl, 2, pts[(b, hl, 2)], [2, 3], direct=d)
                    emit_attnv(b, hl, 0, pts[(b, hl, 0)], [2, 3], direct=d)
                    emit_attnv(b, hl, 2, pts[(b, hl, 2)], [0, 1], direct=d)
                    emit_attnv(b, hl, 0, pts[(b, hl, 0)], [0, 1], direct=d)
                    if k + 2 < len(pairs):
                        emit_bc_scores(k + 2)

                flush_fin()
                # collectives first: their triggers drain program-order-prior
                # DMA work, so nothing else may be emitted before them
                nc.gpsimd.collective_compute(
                    "AllToAll", mybir.AluOpType.bypass,
                    replica_groups=[list(range(W))],
                    ins=[a2a_in['B'][:].opt()], outs=[a2a_out['B'][:].opt()],
                )

                lhs_load(lhsA, 'A', 4)

                def s4_pass(lhs, m0, nm, wbp, ncx0=0, clear_scalar_tail=False):
                    for ncx in range(ncx0, NS4):
                        if ncx + 3 < NS4:
                            wbp[ncx + 3] = load_wb(ncx + 3, last_write['B'])
                        # keep the scalar DGE queue free of stores near the
                        # end of pass A: the lhsB loads land on it at a2a-B
                        # completion and must not queue behind ~5us of
                        # DIRECT2D output stores (they gate s4B on every
                        # core, 1:1 on the kernel's critical path)
                        oe = nc.sync if (clear_scalar_tail
                                         and ncx >= NS4 - 2) else None
                        emit_s4_block(lhs, m0, 0, nm, ncx, wbp.pop(ncx),
                                      out_eng=oe)

                s4_pass(lhsA, 4, 4, wbs, clear_scalar_tail=True)
                # lhsB loads emitted only now: they wait on a2a-B completion,
                # and anything queued behind them on the scalar DGE queue
                # (half the osb stores) would wedge with them if emitted
                # before pass A.
                lhs_load(lhsB, 'B', 4)
                s4_pass(lhsB, 0, 4, {0: load_wb(0, last_write['B']),
                                     1: load_wb(1, last_write['B']),
                                     2: load_wb(2, last_write['B'])})

    nc.compile()
    return nc


_NC = None


def _get_nc():
    global _NC
    if _NC is None:
        _NC = build()
    return _NC


last_exec_time_ns = None
last_results = None


def kernel(x, w_qkv, w_out, b_out):
    global last_exec_time_ns, last_results
    _install_ntff_hook()
    nc = _get_nc()

    x = np.asarray(x, dtype=np.float32)
    w_qkv = np.asarray(w_qkv, dtype=np.float32)
    w_out = np.asarray(w_out, dtype=np.float32)
    b_out = np.asarray(b_out, dtype=np.float32)

    bf = ml_dtypes.bfloat16
    xT = np.ascontiguousarray(x.reshape(ROWS, DIM).T.astype(bf))
    wout_b = np.ascontiguousarray(w_out.astype(bf))
    bout2 = np.ascontiguousarray(b_out.reshape(1, DIM))

    in_maps = []
    for core in range(W):
        cols = [w_qkv[:, part * (HEADS * D) + core * HPC * D:
                      part * (HEADS * D) + (core + 1) * HPC * D]
                for part in range(3)]
        cols[0] = cols[0] * SCALE  # fold softmax scale into q projection
        wq_c = np.ascontiguousarray(np.concatenate(cols, axis=1).astype(bf))
        in_maps.append({"xT": xT, "wq": wq_c, "wout": wout_b, "bout": bout2})

    trace = os.environ.get("KERNEL_TRACE", "") not in ("", "0")
    res = bass_utils.run_bass_kernel_spmd(
        nc, in_maps, core_ids=list(range(W)), trace=trace)
    last_exec_time_ns = res.exec_time_ns
    last_results = res

    out = np.concatenate([res.results[c]["out"] for c in range(W)], axis=0)
    return np.ascontiguousarray(out.reshape(B, N, DIM), dtype=np.float32)



# revision 52
# speedup vs baseline: 1.0064x; 1.0055x over previous
"""Trainium2 Bass kernel for nn_Attention_23476291240422 (sparse attention:
causal + 128-wide noncausal prefix block; b=4, n=2048, dim=2048, 16 heads,
d=128) distributed across 8 NeuronCores.

v4e design for gapless TensorE (PE sustains ~1.9 GHz; the kernel is row-count
bound at ~1.33M matmul rows/core, so every PE idle ns is pure loss):
- [inner, rows] AllToAll payloads; the [i,d]->[d,i] flip runs on TensorE,
  one transpose per i-tile, staged into per-chunk [P,4,P] buffers so each
  payload store is ONE contiguous 128KB write (SBUF->DRAM stores serialize
  as DIRECT2D on the issuing sequencer at ~56GB/s — write count matters).
- 2 collective phases by query-position: phase A = query chunks 1,3, phase
  B = chunks 0,2. Scope 1 computes stage-1 + ALL phase-A attention, then
  a2a-A fires and hides under scope 2's phase-B (BC) attention. a2a-B then
  hides under stage-4 pass A (lhsA only needs a2a-A). lhs loads are chunked
  DMAs on the scalar (Activation) DGE queue, pinned after the last exp;
  lhsB's loads are emitted only after pass A so nothing queues behind their
  collective wait. Collectives have a ~20us bulk phase that monopolizes all
  16 DMA engines: everything overlapping a collective is preloaded (va
  bufs=5, wb prefetch distance 3) or buffered (s4o bufs=8, fin bufs=6).
- Startup: kt-quartered accumulation for the first s1 chunk — the first
  matmul needs only 1.25MB (first wq+xb quarters), loads interleaved across
  both HWDGE queues in consumption order; bout load deferred to scope 2.
- Stage-4 in 256-col chunks, output stores alternating sync/scalar
  sequencers (one sequencer alone cannot keep up with stage-4's rate).
- Deep software pipelining: phase-A attention of batch b interleaved inside
  stage-1 of batch b+1; scores of the next (batch,head) pair emitted before
  attn@v of the current one so scalar-engine exp latency is hidden; xb
  prefetched one chunk ahead; causal diagonal masks on the idle gpsimd.
- Softmax scale folded into the q columns of w_qkv on the host.
"""
import os
import sys
import types

import numpy as np
import ml_dtypes

import concourse.bass as bass
import concourse.mybir as mybir
import concourse.tile as tile
from concourse.tile import add_dep_helper
from concourse import bacc
from concourse import bass_utils

B, N, DIM = 4, 2048, 2048
HEADS, D, L = 16, 128, 128
W = 8
HPC = HEADS // W          # 2 heads per core
ROWS = B * N              # 8192
RPC = ROWS // W           # 1024 rows per core
SCALE = float(D) ** -0.5
P = 128
KT = DIM // P             # 16
S1CH = 512                # stage-1 seq chunk width
NBC = N // S1CH           # 4 stage-1 chunks per batch
CHW = 512                 # score chunk width
NJT = N // P              # 16 j-tiles
F32, BF16 = mybir.dt.float32, mybir.dt.bfloat16

# attention chunks (512 queries each): phase A = full chunks 1,3
# (it%8 in 4..7 -> dest rows 512-1023); chunks 0,2 split by pp:
# pp 2,3 -> phase B (rows 256-511), pp 0,1 -> phase C (rows 0-255).
PT_NJ = {0: 4, 1: 8, 2: 12, 3: 16}   # j-tiles needed per chunk


def _install_ntff_hook():
    try:
        import antenv.axon_hooks  # noqa: F401
        return
    except ImportError:
        pass
    try:
        import antenv
        from trn_agent_boot.trn_boot import _ntff_profile_via_ctypes
        hook = [_ntff_profile_via_ctypes("/opt/axon/libaxon_pjrt.so")]
        mod = types.ModuleType("antenv.axon_hooks")
        mod.get_axon_ntff_profile_hook = lambda: hook[0]
        mod.set_axon_ntff_profile_hook = lambda h: hook.__setitem__(0, h)
        sys.modules["antenv.axon_hooks"] = mod
        antenv.axon_hooks = mod
    except Exception:
        pass


def build():
    nc = bacc.Bacc("TRN2", target_bir_lowering=False, debug=False, num_devices=W)

    xT = nc.dram_tensor("xT", [DIM, ROWS], BF16, kind="ExternalInput")
    wq = nc.dram_tensor("wq", [DIM, 6 * P], BF16, kind="ExternalInput")  # q0 q1 k0 k1 v0 v1 (q pre-scaled)
    wout = nc.dram_tensor("wout", [DIM, DIM], BF16, kind="ExternalInput")
    bout = nc.dram_tensor("bout", [1, DIM], F32, kind="ExternalInput")
    out = nc.dram_tensor("out", [RPC, DIM], F32, kind="ExternalOutput")

    tri_np = (np.arange(P)[:, None] <= np.arange(P)[None, :]).astype(ml_dtypes.bfloat16)
    tri = nc.inline_tensor(tri_np, name="tri")
    ident = nc.inline_tensor(np.eye(P, dtype=ml_dtypes.bfloat16), name="ident")

    with tile.TileContext(nc) as tc:
        with (
            tc.tile_pool(name="persist", bufs=1) as persist,
            tc.tile_pool(name="dram", bufs=1, space="DRAM") as dram,
            tc.tile_pool(name="qk", bufs=1) as qkpool,
            tc.tile_pool(name="va", bufs=5) as vapool,
            tc.tile_pool(name="sm", bufs=8) as smpool,
            tc.tile_pool(name="fin", bufs=6) as finpool,
            tc.tile_pool(name="pss", bufs=2, space="PSUM") as psspool,
            tc.tile_pool(name="att", bufs=2, space="PSUM") as attpool,
            tc.tile_pool(name="s2tp", bufs=2, space="PSUM") as tppool,
        ):
            tri_sb = persist.tile([P, P], BF16)
            ident_sb = persist.tile([P, P], BF16)
            bout_sb = persist.tile([P, DIM], F32)

            v_drams = [dram.tile([N, HPC * P], BF16, name=f"v_dram{b}")
                       for b in range(B)]
            a2a_in = {
                'A': dram.tile([W, HPC * P, 4 * P], BF16, name="a2aA_in"),
                'B': dram.tile([W, HPC * P, 4 * P], BF16, name="a2aB_in"),
            }
            a2a_out = {
                'A': dram.tile([W, HPC * P, 4 * P], BF16, name="a2aA_out"),
                'B': dram.tile([W, HPC * P, 4 * P], BF16, name="a2aB_out"),
            }

            woutb_r = wout.rearrange("(kt p) c -> p kt c", p=P)
            qk_bs = [qkpool.tile([P, 4, N], BF16, name=f"qkb{b}")
                     for b in range(B)]
            va_tiles = {}

            def load_va(b, hl):
                va = vapool.tile([P, NJT, P + 1], BF16, tag="va")
                nc.vector.memset(va[:, :, P:P + 1], 1.0)
                v_r = v_drams[b].rearrange("(jt p) d -> p jt d", p=P)
                nc.sync.dma_start(va[:, :, :P], v_r[:, :, hl * P:(hl + 1) * P])
                va_tiles[(b, hl)] = va

            last_exp = [None]  # most recent exp instruction (for load pinning)

            def emit_scores(b, hl, c, pool, tag, alloc_nj=None, dve_c0=False):
                """Compute pt = exp(scores) for chunk c of (b, hl); also
                prefetches va for the later attn@v. alloc_nj lets a smaller
                chunk borrow a larger tag's slot shape."""
                if (b, hl) not in va_tiles:
                    load_va(b, hl)
                nj = PT_NJ[c]
                pt = pool.tile([P, alloc_nj or nj, CHW], BF16, tag=tag)
                qT = qk_bs[b][:, hl]
                kTt = qk_bs[b][:, 2 + hl]
                for J in range(nj):
                    k_off = max(0, J - 4 * c)
                    nn_ = CHW - P * k_off
                    i0 = c * CHW + P * k_off
                    pss = psspool.tile([P, CHW], F32, tag="pss")
                    nc.tensor.matmul(
                        pss[:, :nn_], kTt[:, J * P:(J + 1) * P],
                        qT[:, i0:(c + 1) * CHW], start=True, stop=True)
                    last_exp[0] = nc.scalar.activation(
                        pt[:, J, P * k_off:], pss[:, :nn_],
                        mybir.ActivationFunctionType.Exp)
                    if J >= 4 * c and not (c == 0 and J == 0):
                        # diagonal-tile causal mask on the (idle) gpsimd
                        # engine, freeing DVE cycles
                        nc.gpsimd.tensor_mul(
                            pt[:, J, P * k_off:P * (k_off + 1)],
                            pt[:, J, P * k_off:P * (k_off + 1)], tri_sb[:])
                return pt

            # deferred transpose+write: holds (attn_tile, ph, dest, hl, r0)
            # for the 2 most recent attn@v outputs; flushed two i-tiles later
            # so the TensorE transpose never waits on the vector scale even
            # for short chains. Transposed tiles are staged into a per-chunk
            # [P, 4, P] buffer; when all 4 land, ONE contiguous 128KB DRAM
            # write goes out (stores serialize as DIRECT2D on the sequencer,
            # so fewer/bigger writes keep the a2a triggers prompt).
            pending_fin = []
            last_write = {'A': None, 'B': None}
            fin_bufs = {}  # (ph, dest, hl) -> [tile, count]

            def fin_one(item):
                attn, ph, dest, hl, r0, direct = item
                attT = tppool.tile([P, P], BF16, tag="attT")
                nc.tensor.transpose(attT[:], attn[:], ident_sb[:])
                if direct:
                    # per-tile write: lands right after its copy, skipping
                    # the 4-tile staging + big-write latency (used for the
                    # last BC pair, whose tail gates the a2a-B trigger).
                    # Borrows a finpool slot per tile to avoid a new SBUF tag.
                    dt_t = finpool.tile([P, 4, P], BF16, tag="attnC",
                                        name=f"attnD_{ph}_{dest}_{hl}_{r0}")
                    nc.vector.tensor_copy(dt_t[:, 0], attT[:])
                    last_write[ph] = nc.sync.dma_start(
                        a2a_in[ph][dest, hl * P:(hl + 1) * P, r0:r0 + P],
                        dt_t[:, 0])
                    return
                key = (ph, dest, hl)
                if key not in fin_bufs:
                    ac_t = finpool.tile([P, 4, P], BF16, tag="attnC",
                                        name=f"attnC_{ph}_{dest}_{hl}")
                    fin_bufs[key] = [ac_t, 0]
                ac = fin_bufs[key]
                nc.vector.tensor_copy(ac[0][:, r0 // P], attT[:])
                ac[1] += 1
                if ac[1] == 4:
                    last_write[ph] = nc.sync.dma_start(
                        a2a_in[ph][dest, hl * P:(hl + 1) * P, :],
                        ac[0][:])
                    del fin_bufs[key]

            def flush_fin():
                while pending_fin:
                    fin_one(pending_fin.pop(0))
                assert not fin_bufs, f"incomplete payload chunks: {list(fin_bufs)}"

            def emit_attnv(b, hl, c, pt, pp_list, direct=False):
                """attn@v + normalize for the given pp's of chunk c; the
                transpose+payload-write of each tile is deferred by one."""
                va = va_tiles[(b, hl)]
                for pp in pp_list:
                    it = 4 * c + pp
                    att = attpool.tile([P, P + 1], F32, tag="att")
                    for J in range(it + 1):
                        nc.tensor.matmul(
                            att[:], pt[:, J, P * pp:P * (pp + 1)], va[:, J],
                            start=(J == 0), stop=(J == it))
                    recip = smpool.tile([P, 1], F32, tag="recip")
                    nc.vector.reciprocal(recip[:], att[:, P:P + 1])
                    attn = smpool.tile([P, P], BF16, tag="attn")
                    nc.vector.tensor_scalar_mul(attn[:], att[:, :P], recip[:])
                    dest = b * 2 + it // 8
                    rel = it % 8
                    if rel >= 4:
                        ph, r0 = 'A', (rel - 4) * P
                    else:
                        ph, r0 = 'B', rel * P
                    pending_fin.append((attn, ph, dest, hl, r0, direct))
                    while len(pending_fin) > 2:
                        fin_one(pending_fin.pop(0))

            # ================= scope 1: stage 1 + phase-A attention =========
            pts = {}
            with (
                tc.tile_pool(name="s1w", bufs=1) as s1w,
                tc.tile_pool(name="s1v", bufs=3) as s1v_pool,
                tc.tile_pool(name="s1xf", bufs=2) as s1xf,
                tc.tile_pool(name="s1ps", bufs=2, space="PSUM") as s1ps,
                tc.tile_pool(name="ptA", bufs=2) as ptA,
            ):
                wq_bf = s1w.tile([P, KT, 6 * P], BF16)
                wq_r = wq.rearrange("(kt p) c -> p kt c", p=P)
                xT_r = xT.rearrange("(kt p) n -> p kt n", p=P)
                xb_tiles = {}

                def prefetch_xb(b, c, split):
                    xb = s1xf.tile([P, KT, S1CH], BF16, tag="xb")
                    seq0 = b * N + c * S1CH
                    if split:
                        for kq in range(4):
                            # quarter 0 first on the (otherwise empty) scalar
                            # queue so the kt-split first matmul starts early
                            eng = nc.scalar if kq % 2 == 0 else nc.sync
                            eng.dma_start(
                                xb[:, 4 * kq:4 * (kq + 1)],
                                xT_r[:, 4 * kq:4 * (kq + 1), seq0:seq0 + S1CH])
                    else:
                        nc.sync.dma_start(xb[:], xT_r[:, :, seq0:seq0 + S1CH])
                    xb_tiles[(b, c)] = xb

                def emit_s1_chunk(b, c):
                    # prefetch next chunk's x before computing this one
                    nxt = b * NBC + c + 1
                    if nxt < B * NBC:
                        prefetch_xb(nxt // NBC, nxt % NBC, split=False)
                    xb = xb_tiles.pop((b, c))
                    if (b, c) == (0, 0):
                        # kt-quartered accumulation: the first matmul needs
                        # only the first wq/xb quarter (1.25MB), not the full
                        # 5MB — PE starts ~5us earlier
                        for mg in range(2):
                            psp = [s1ps.tile([P, CHW], F32, tag="ps1",
                                             name=f"ps1q{mg}{i}")
                                   for i in range(2)]
                            for ktq in range(4):
                                for mi in range(2):
                                    for kt in range(4 * ktq, 4 * ktq + 4):
                                        nc.tensor.matmul(
                                            psp[mi][:, :S1CH],
                                            wq_bf[:, kt,
                                                  (2 * mg + mi) * P:
                                                  (2 * mg + mi + 1) * P],
                                            xb[:, kt],
                                            start=(kt == 0),
                                            stop=(kt == KT - 1))
                            for mi in range(2):
                                nc.vector.tensor_copy(
                                    qk_bs[b][:, 2 * mg + mi, :S1CH],
                                    psp[mi][:, :S1CH])
                    else:
                        for m in range(4):
                            ps = s1ps.tile([P, CHW], F32, tag="ps1")
                            for kt in range(KT):
                                nc.tensor.matmul(
                                    ps[:, :S1CH],
                                    wq_bf[:, kt, m * P:(m + 1) * P],
                                    xb[:, kt],
                                    start=(kt == 0), stop=(kt == KT - 1))
                            nc.vector.tensor_copy(
                                qk_bs[b][:, m, c * S1CH:(c + 1) * S1CH],
                                ps[:, :S1CH])
                    for st2 in range(S1CH // P):
                        st = c * (S1CH // P) + st2
                        psv = s1ps.tile([P, CHW], F32, tag="ps1")
                        for kt in range(KT):
                            nc.tensor.matmul(
                                psv[:, :HPC * P], xb[:, kt, st2 * P:(st2 + 1) * P],
                                wq_bf[:, kt, 4 * P:6 * P],
                                start=(kt == 0), stop=(kt == KT - 1))
                        vst = s1v_pool.tile([P, HPC * P], BF16, tag="vst")
                        nc.vector.tensor_copy(vst[:], psv[:, :HPC * P])
                        nc.sync.dma_start(
                            v_drams[b][st * P:(st + 1) * P, :], vst[:])

                # startup loads, interleaved across both HWDGE queues in the
                # exact order the kt-quartered first chunk consumes them:
                # ktq0 needs wq[0:4] (sync) + xb q0 (scalar) = 1.25MB only
                xb00 = s1xf.tile([P, KT, S1CH], BF16, tag="xb")
                nc.sync.dma_start(wq_bf[:, 0:4], wq_r[:, 0:4])
                nc.scalar.dma_start(xb00[:, 0:4], xT_r[:, 0:4, :S1CH])
                nc.sync.dma_start(xb00[:, 4:8], xT_r[:, 4:8, :S1CH])
                nc.scalar.dma_start(wq_bf[:, 4:8], wq_r[:, 4:8])
                nc.sync.dma_start(wq_bf[:, 8:12], wq_r[:, 8:12])
                nc.scalar.dma_start(xb00[:, 8:12], xT_r[:, 8:12, :S1CH])
                nc.sync.dma_start(xb00[:, 12:16], xT_r[:, 12:16, :S1CH])
                nc.scalar.dma_start(wq_bf[:, 12:16], wq_r[:, 12:16])
                xb_tiles[(0, 0)] = xb00
                # mask/identity tiles: small loads queued behind the first
                # chunk (first needed at batch-0 scores, ~130us in)
                nc.sync.dma_start(tri_sb[:], tri.ap())
                nc.scalar.dma_start(ident_sb[:], ident.ap())
                for b in range(B):
                    for c in range(NBC):
                        emit_s1_chunk(b, c)
                        if b >= 1:
                            pb = b - 1
                            if c == 0:
                                emit_attnv(pb, 0, 1, pts[(pb, 0, 1)], [0, 1, 2, 3])
                            elif c == 1:
                                emit_attnv(pb, 0, 3, pts[(pb, 0, 3)], [0, 1, 2, 3])
                                del va_tiles[(pb, 0)]  # phase-A use done
                            elif c == 2:
                                pts[(pb, 1, 1)] = emit_scores(pb, 1, 1, ptA, "ptc1")
                                pts[(pb, 1, 3)] = emit_scores(pb, 1, 3, ptA, "ptc3")
                            else:
                                emit_attnv(pb, 1, 1, pts[(pb, 1, 1)], [0, 1, 2, 3])
                                emit_attnv(pb, 1, 3, pts[(pb, 1, 3)], [0, 1, 2, 3])
                                del va_tiles[(pb, 1)]  # phase-A use done
                    pts[(b, 0, 1)] = emit_scores(b, 0, 1, ptA, "ptc1")
                    pts[(b, 0, 3)] = emit_scores(b, 0, 3, ptA, "ptc3")

                # tail: batch 3 phase-A attention (no s1 left to hide under)
                b = B - 1
                pts[(b, 1, 1)] = emit_scores(b, 1, 1, ptA, "ptc1")
                emit_attnv(b, 0, 1, pts[(b, 0, 1)], [0, 1, 2, 3])
                pts[(b, 1, 3)] = emit_scores(b, 1, 3, ptA, "ptc3")
                emit_attnv(b, 0, 3, pts[(b, 0, 3)], [0, 1, 2, 3])
                emit_attnv(b, 1, 1, pts[(b, 1, 1)], [0, 1, 2, 3])
                emit_attnv(b, 1, 3, pts[(b, 1, 3)], [0, 1, 2, 3])
                flush_fin()

            nc.gpsimd.collective_compute(
                "AllToAll", mybir.AluOpType.bypass,
                replica_groups=[list(range(W))],
                ins=[a2a_in['A'][:].opt()], outs=[a2a_out['A'][:].opt()],
            )

            # ============== scope 2: BC attention + stage 4 =================
            with (
                tc.tile_pool(name="ptBC", bufs=2) as ptBC,
                tc.tile_pool(name="s4l", bufs=1) as s4l,
                tc.tile_pool(name="s4w", bufs=4) as s4w,
                tc.tile_pool(name="s4o", bufs=8) as s4o,
                tc.tile_pool(name="s4ps", bufs=2, space="PSUM") as s4ps,
            ):
                lhsA = s4l.tile([P, KT, 4 * P], BF16, name="lhsA")
                lhsB = s4l.tile([P, KT, 4 * P], BF16, name="lhsB")

                def lhs_load(dst, ph, nsplit):
                    # plain chunked DMA on the scalar (Activation) DGE queue:
                    # pinned after the last exp so the static scheduler cannot
                    # hoist it ahead of pending exps (its collective wait
                    # would stall them); waits there blocking nothing.
                    # Split along the m/row dim so the first s4 block of the
                    # pass can start after the first quarter arrives.
                    view = (a2a_out[ph].rearrange("w h r -> (w h) r")
                            .rearrange("(kt p) r -> p kt r", p=P))
                    step = (4 * P) // nsplit
                    for q in range(nsplit):
                        d = nc.scalar.dma_start(
                            dst[:, :, q * step:(q + 1) * step],
                            view[:, :, q * step:(q + 1) * step])
                        add_dep_helper(d.ins, last_exp[0].ins, sync=False,
                                       reason="lhs load after all exps")

                S4C = 256          # stage-4 column chunk
                NS4 = DIM // S4C   # 8 chunks

                def load_wb(ncx, pin):
                    # wout chunk load, split in two; optionally pinned after
                    # a payload write so the scheduler can't hoist it into
                    # the BC stream's DMA window
                    wb = s4w.tile([P, KT, S4C], BF16, tag="wb")
                    for h in range(2):
                        d = nc.sync.dma_start(
                            wb[:, 8 * h:8 * (h + 1)],
                            woutb_r[:, 8 * h:8 * (h + 1),
                                    ncx * S4C:(ncx + 1) * S4C])
                        if pin is not None:
                            add_dep_helper(d.ins, pin.ins, sync=False,
                                           reason="wb after attn writes")
                    return wb

                def emit_s4_block(lhs, m0, lc0, nm, ncx, wb, out_eng=None):
                    for ml in range(nm):
                        m = m0 + ml
                        lc = lc0 + ml
                        ps4 = s4ps.tile([P, S4C], F32, tag="ps4")
                        for kt in range(KT):
                            nc.tensor.matmul(
                                ps4[:], lhs[:, kt, lc * P:(lc + 1) * P],
                                wb[:, kt],
                                start=(kt == 0), stop=(kt == KT - 1))
                        osb = s4o.tile([P, S4C], F32, tag="osb")
                        nc.vector.tensor_tensor(
                            osb[:], ps4[:],
                            bout_sb[:, ncx * S4C:(ncx + 1) * S4C],
                            mybir.AluOpType.add)
                        # alternate sequencers: DIRECT2D stores run at
                        # ~56GB/s serialized per queue; one queue alone
                        # cannot keep up with stage-4's output rate
                        eng = out_eng or (
                            nc.sync if (m + ncx) % 2 == 0 else nc.scalar)
                        eng.dma_start(
                            out[m * P:(m + 1) * P,
                                ncx * S4C:(ncx + 1) * S4C],
                            osb[:])

                # bias + out-proj weight chunks 0-3 preloaded early (no deps)
                nc.sync.dma_start(
                    bout_sb[:], bout.ap().to_broadcast((P, DIM)))
                wbs = {n: load_wb(n, None) for n in range(4)}

                # ALL phase-B (BC) attention runs here, hiding a2a-A
                pairs = [(b, hl) for b in range(B) for hl in range(HPC)]
                va_tiles.clear()  # phase-A va slots are stale; reload per pair

                def emit_bc_scores(k):
                    b, hl = pairs[k]
                    pts[(b, hl, 0)] = emit_scores(b, hl, 0, ptBC, "ptc0",
                                                  dve_c0=True)
                    pts[(b, hl, 2)] = emit_scores(b, hl, 2, ptBC, "ptc2")

                emit_bc_scores(0)
                emit_bc_scores(1)
                last_k = len(pairs) - 1
                for k, (b, hl) in enumerate(pairs):
                    # big chunk (c2) first, tiny c0 chain last: the slowest
                    # core's a2a-B trigger waits on the tail of this chain,
                    # so end each pair with the shortest dependency tail.
                    # The last pair writes per-tile (not chunk-batched) so
                    # its payload lands with minimal staging latency.
                    d = (k == last_k)
                    emit_attnv(b, hl, 2, pts[(b, hl, 2)], [2, 3], direct=d)
                    emit_attnv(b, hl, 0, pts[(b, hl, 0)], [2, 3], direct=d)
                    emit_attnv(b, hl, 2, pts[(b, hl, 2)], [0, 1], direct=d)
                    emit_attnv(b, hl, 0, pts[(b, hl, 0)], [0, 1], direct=d)
                    if k + 2 < len(pairs):
                        emit_bc_scores(k + 2)

                flush_fin()
                # collectives first: their triggers drain program-order-prior
                # DMA work, so nothing else may be emitted before them
                nc.gpsimd.collective_compute(
                    "AllToAll", mybir.AluOpType.bypass,
                    replica_groups=[list(range(W))],
                    ins=[a2a_in['B'][:].opt()], outs=[a2a_out['B'][:].opt()],
                )

                lhs_load(lhsA, 'A', 4)

                def s4_pass(lhs, m0, nm, wbp, ncx0=0, clear_scalar_tail=False):
                    for ncx in range(ncx0, NS4):
                        if ncx + 3 < NS4:
                            wbp[ncx + 3] = load_wb(ncx + 3, last_write['B'])
                        # keep the scalar DGE queue free of stores near the
                        # end of pass A: the lhsB loads land on it at a2a-B
                        # completion and must not queue behind ~5us of
                        # DIRECT2D output stores (they gate s4B on every
                        # core, 1:1 on the kernel's critical path)
                        oe = nc.sync if (clear_scalar_tail
                                         and ncx >= NS4 - 2) else None
                        emit_s4_block(lhs, m0, 0, nm, ncx, wbp.pop(ncx),
                                      out_eng=oe)

                s4_pass(lhsA, 4, 4, wbs, clear_scalar_tail=True)
                # lhsB loads emitted only now: they wait on a2a-B completion,
                # and anything queued behind them on the scalar DGE queue
                # (half the osb stores) would wedge with them if emitted
                # before pass A.
                lhs_load(lhsB, 'B', 4)
                s4_pass(lhsB, 0, 4, {0: load_wb(0, last_write['B']),
                                     1: load_wb(1, last_write['B']),
                                     2: load_wb(2, last_write['B'])})

    nc.compile()
    return nc


_NC = None


def _get_nc():
    global _NC
    if _NC is None:
        _NC = build()
    return _NC


last_exec_time_ns = None
last_results = None


def kernel(x, w_qkv, w_out, b_out):
    global last_exec_time_ns, last_results
    _install_ntff_hook()
    nc = _get_nc()

    x = np.asarray(x, dtype=np.float32)
    w_qkv = np.asarray(w_qkv, dtype=np.float32)
    w_out = np.asarray(w_out, dtype=np.float32)
    b_out = np.asarray(b_out, dtype=np.float32)

    bf = ml_dtypes.bfloat16
    xT = np.ascontiguousarray(x.reshape(ROWS, DIM).T.astype(bf))
    wout_b = np.ascontiguousarray(w_out.astype(bf))
    bout2 = np.ascontiguousarray(b_out.reshape(1, DIM))

    in_maps = []
    for core in range(W):
        cols = [w_qkv[:, part * (HEADS * D) + core * HPC * D:
                      part * (HEADS * D) + (core + 1) * HPC * D]
                for part in range(3)]
        cols[0] = cols[0] * SCALE  # fold softmax scale into q projection
        wq_c = np.ascontiguousarray(np.concatenate(cols, axis=1).astype(bf))
        in_maps.append({"xT": xT, "wq": wq_c, "wout": wout_b, "bout": bout2})

    trace = os.environ.get("KERNEL_TRACE", "") not in ("", "0")
    res = bass_utils.run_bass_kernel_spmd(
        nc, in_maps, core_ids=list(range(W)), trace=trace)
    last_exec_time_ns = res.exec_time_ns
    last_results = res

    out = np.concatenate([res.results[c]["out"] for c in range(W)], axis=0)
    return np.ascontiguousarray(out.reshape(B, N, DIM), dtype=np.float32)

